# revision 1
# baseline (speedup 1.0000x reference)
"""Trainium2 Bass kernel for nn_MHA_43095701848407.

MHA forward: qkv = x @ W_qkv, RoPE on q/k, causal softmax attention,
y @ W_proj.  B=4, T=2048, C=2048, 16 heads, head_dim=128, fp32.

Sharding (8 cores): tensor-parallel over heads (4 shards x 4 heads) x
data-parallel over batch (2 groups x 2 batches).  core = group*4 + shard.
Each core computes, for its 2 batches and 4 heads:
  qkv^T tiles via fp32r matmuls (x^T streamed, W resident),
  RoPE via a permutation matmul + vector combines,
  causal attention in transposed orientation (scores^T [k,q]; exp on ACT;
  column sums via ones-matmul; y^T = v_nat.T @ p^T), then the local slice
  of the output projection, producing a partial out^T [C, T] per batch.
Host sums the 4 head-shard partials per batch and transposes back.

Self-contained: shapes/sharding hardcoded; inputs full-size numpy arrays.
"""

import math
import os
import sys
import types

import numpy as np

import concourse.bass as bass
import concourse.mybir as mybir
import concourse.tile as tile
from concourse import bacc
from concourse.bass_utils import run_bass_kernel_spmd

F32 = mybir.dt.float32
F32R = mybir.dt.float32r
AF = mybir.ActivationFunctionType
ALU = mybir.AluOpType

# Problem shape (hardcoded per contract)
B, T, C = 4, 2048, 2048
H, HD = 16, 128
NCORES = 8
BGROUPS, HSHARDS = 2, 4  # batch groups x head shards
B_LOC = B // BGROUPS  # 2 batches per core
H_LOC = H // HSHARDS  # 4 heads per core
FQK = H_LOC * HD  # 512 features for q (and for k)
FV = H_LOC * HD  # 512 features for v
F_ALL = 3 * H_LOC * HD  # 1536 qkv features per core
KO = C // 128  # 16 contraction chunks
TSLAB = 512
NSLAB = T // TSLAB  # 4 t-slabs per batch
QTILE = 512
NQT = T // QTILE  # 4 q-tiles
NKB = T // 128  # 16 key blocks
SCALE = 1.0 / math.sqrt(HD)

_CACHED = {}


def _install_ntff_hook():
    """Register the axon NTFF profile hook (container's antenv lacks it)."""
    if "antenv.axon_hooks" in sys.modules:
        return
    try:
        mod = types.ModuleType("antenv.axon_hooks")
        holder = [None]
        mod.set_axon_ntff_profile_hook = lambda h: holder.__setitem__(0, h)
        mod.get_axon_ntff_profile_hook = lambda: holder[0]
        sys.modules["antenv.axon_hooks"] = mod
        import antenv

        antenv.axon_hooks = mod
        if "/root/.axon_site" not in sys.path:
            sys.path.insert(0, "/root/.axon_site")
        from trn_agent_boot.trn_boot import _ntff_profile_via_ctypes

        mod.set_axon_ntff_profile_hook(
            _ntff_profile_via_ctypes("/opt/axon/libaxon_pjrt.so")
        )
    except Exception:
        sys.modules.pop("antenv.axon_hooks", None)


def rope_perm_matrix():
    """lhsT for the rotate-half matmul: rot^T = PT.T @ q^T.
    rot[2i] = -q[2i+1], rot[2i+1] = q[2i]."""
    pt = np.zeros((HD, HD), dtype=np.float32)
    for i in range(HD // 2):
        pt[2 * i + 1, 2 * i] = -1.0
        pt[2 * i, 2 * i + 1] = 1.0
    return pt


def build_nc():
    nc = bacc.Bacc("TRN2", target_bir_lowering=False, debug=False)

    x_t = nc.dram_tensor("x_t", [B_LOC, C, T], F32R, kind="ExternalInput").ap()
    w_qkv = nc.dram_tensor("w_qkv", [C, F_ALL], F32R, kind="ExternalInput").ap()
    w_proj = nc.dram_tensor("w_proj", [FV, C], F32R, kind="ExternalInput").ap()
    sin_t = nc.dram_tensor("sin_t", [HD, T], F32, kind="ExternalInput").ap()
    cos_t = nc.dram_tensor("cos_t", [HD, T], F32, kind="ExternalInput").ap()
    pt = nc.dram_tensor("pt", [HD, HD], F32R, kind="ExternalInput").ap()
    ones_col = nc.dram_tensor("ones_col", [128, 1], F32R, kind="ExternalInput").ap()
    ones_row = nc.dram_tensor("ones_row", [1, 128], F32R, kind="ExternalInput").ap()
    out_t = nc.dram_tensor("out_t", [B_LOC, C, T], F32, kind="ExternalOutput").ap()

    with tile.TileContext(nc) as tc:
        with nc.allow_low_precision(reason="fp32r matmul inputs by design"):
            _emit(nc, tc, x_t, w_qkv, w_proj, sin_t, cos_t, pt, ones_col,
                  ones_row, out_t)
    nc.compile()
    return nc


def _emit(nc, tc, x_t, w_qkv, w_proj, sin_t, cos_t, pt, ones_col, ones_row, out_t):
    # ---- persistent scratch in DRAM ----
    with tc.tile_pool(name="dram", bufs=1, space="DRAM") as dram_pool:
        qk_dram = [
            dram_pool.tile([2 * FQK, T], F32R, name=f"qk_dram{b}") for b in range(B_LOC)
        ]
        v_dram = [
            dram_pool.tile([T, FV], F32R, name=f"v_dram{b}") for b in range(B_LOC)
        ]

        with tc.tile_pool(name="consts", bufs=1) as consts:
            pt_sb = consts.tile([HD, HD], F32R)
            nc.sync.dma_start(pt_sb, pt)
            ones_c_sb = consts.tile([128, 1], F32R)
            nc.sync.dma_start(ones_c_sb, ones_col)
            ones_r_sb = consts.tile([1, 128], F32R)
            nc.sync.dma_start(ones_r_sb, ones_row)

            _phase_qkv(nc, tc, x_t, w_qkv, sin_t, cos_t, pt_sb, qk_dram, v_dram)
            _phase_attn_proj(
                nc, tc, w_proj, qk_dram, v_dram, ones_c_sb, ones_r_sb, out_t
            )


def _phase_qkv(nc, tc, x_t, w_qkv, sin_t, cos_t, pt_sb, qk_dram, v_dram):
    """qkv^T = W.T @ x^T with RoPE on q,k; v in natural [t, f] layout."""
    with (
        tc.tile_pool(name="wpool", bufs=1) as wpool,
        tc.tile_pool(name="xpool", bufs=2) as xpool,
        tc.tile_pool(name="scpool", bufs=2) as scpool,
        tc.tile_pool(name="ropepool", bufs=2) as ropepool,
        tc.tile_pool(name="qkpsum", bufs=3, space="PSUM") as qkpsum,
        tc.tile_pool(name="rotpsum", bufs=2, space="PSUM") as rotpsum,
        tc.tile_pool(name="vpsum", bufs=2, space="PSUM") as vpsum,
    ):
        w_sb = wpool.tile([128, KO, F_ALL], F32R)
        w_src = w_qkv.rearrange("(ko p) f -> p ko f", p=128)
        for ko in range(KO):
            nc.scalar.dma_start(w_sb[:, ko, :], w_src[:, ko, :])

        for b in range(B_LOC):
            x3 = x_t[b].rearrange("(ko p) t -> p ko t", p=128)
            for js in range(NSLAB):
                first = b == 0 and js == 0
                tsl = slice(js * TSLAB, (js + 1) * TSLAB)
                x_sb = xpool.tile([128, KO, TSLAB], F32R, name="x_sb")
                if first:
                    # split by ko so the first matmuls start after ~1/16 load
                    for ko in range(KO):
                        nc.sync.dma_start(x_sb[:, ko, :], x3[:, ko, tsl])
                else:
                    nc.sync.dma_start(x_sb, x3[:, :, tsl])
                sin_sb = scpool.tile([HD, TSLAB], F32, name="sin_sb")
                nc.sync.dma_start(sin_sb, sin_t[:, tsl])
                cos_sb = scpool.tile([HD, TSLAB], F32, name="cos_sb")
                nc.sync.dma_start(cos_sb, cos_t[:, tsl])

                # q^T, k^T feature chunks (heads) with RoPE
                qk_psums = {}
                if first:
                    # ko-outer in two groups of 4 f-chunks: compute proceeds at
                    # W/x chunk-arrival pace instead of waiting for full load
                    for fg in range(2):
                        fs = [fg * 4 + i for i in range(4)]
                        pss = {
                            f: qkpsum.tile([128, TSLAB], F32, name="qk_ps")
                            for f in fs
                        }
                        for ko in range(KO):
                            for f in fs:
                                nc.tensor.matmul(
                                    pss[f],
                                    w_sb[:, ko, f * 128 : (f + 1) * 128],
                                    x_sb[:, ko, :],
                                    start=(ko == 0),
                                    stop=(ko == KO - 1),
                                )
                        qk_psums.update(pss)
                for f in range(2 * H_LOC):
                    if first:
                        ps = qk_psums[f]
                    else:
                        ps = qkpsum.tile([128, TSLAB], F32, name="qk_ps")
                        for ko in range(KO):
                            nc.tensor.matmul(
                                ps,
                                w_sb[:, ko, f * 128 : (f + 1) * 128],
                                x_sb[:, ko, :],
                                start=(ko == 0),
                                stop=(ko == KO - 1),
                            )
                    raw = ropepool.tile([128, TSLAB], F32R, name="raw")
                    nc.vector.tensor_copy(raw, ps)
                    rot_ps = rotpsum.tile([128, TSLAB], F32, name="rot_ps")
                    nc.tensor.matmul(rot_ps, pt_sb, raw, start=True, stop=True)
                    # roped = raw*cos + rot*sin
                    t1 = ropepool.tile([128, TSLAB], F32, name="t1")
                    nc.gpsimd.tensor_tensor(t1, raw, cos_sb, ALU.mult)
                    t2 = ropepool.tile([128, TSLAB], F32, name="t2")
                    nc.vector.tensor_tensor(t2, rot_ps, sin_sb, ALU.mult)
                    roped = ropepool.tile([128, TSLAB], F32R, name="roped")
                    nc.vector.tensor_tensor(roped, t1, t2, ALU.add)
                    nc.sync.dma_start(
                        qk_dram[b][f * 128 : (f + 1) * 128, tsl], roped
                    )

                # v in natural layout
                for tb in range(TSLAB // 128):
                    vps = vpsum.tile([128, FV], F32, name="v_ps")
                    for ko in range(KO):
                        nc.tensor.matmul(
                            vps,
                            x_sb[:, ko, tb * 128 : (tb + 1) * 128],
                            w_sb[:, ko, 2 * FQK : 2 * FQK + FV],
                            start=(ko == 0),
                            stop=(ko == KO - 1),
                        )
                    v_sb = ropepool.tile([128, FV], F32R, name="v_sb")
                    nc.vector.tensor_copy(v_sb, vps)
                    r0 = js * TSLAB + tb * 128
                    nc.sync.dma_start(v_dram[b][r0 : r0 + 128, :], v_sb)


def _phase_attn_proj(nc, tc, w_proj, qk_dram, v_dram, ones_c_sb, ones_r_sb, out_t):
    with (
        tc.tile_pool(name="wppool", bufs=1) as wppool,
        tc.tile_pool(name="qkvload", bufs=3) as qkvload,
        tc.tile_pool(name="ppool", bufs=6) as ppool,
        tc.tile_pool(name="ypool", bufs=B_LOC * H_LOC) as ypool,
        tc.tile_pool(name="npool", bufs=5) as npool,
        tc.tile_pool(name="opool", bufs=3) as opool,
        tc.tile_pool(name="spsum", bufs=3, space="PSUM") as spsum,
        tc.tile_pool(name="ypsum", bufs=2, space="PSUM") as ypsum,
        tc.tile_pool(name="lpsum", bufs=1, space="PSUM") as lpsum,
        tc.tile_pool(name="opsum", bufs=2, space="PSUM") as opsum,
        tc.tile_pool(name="nbounce", bufs=4, space="DRAM") as nbounce,
    ):
        wp_sb = wppool.tile([128, H_LOC, C], F32R)
        nc.sync.dma_start(wp_sb, w_proj.rearrange("(fo p) c -> p fo c", p=128))

        def emit_head_load(b, h):
            qt_sb = qkvload.tile([HD, T], F32R, name="qt_sb")
            nc.scalar.dma_start(qt_sb, qk_dram[b][h * HD : (h + 1) * HD, :])
            kt_sb = qkvload.tile([HD, T], F32R, name="kt_sb")
            nc.scalar.dma_start(
                kt_sb, qk_dram[b][FQK + h * HD : FQK + (h + 1) * HD, :]
            )
            v_sb = qkvload.tile([128, NKB, HD], F32R, name="v_sb")
            nc.scalar.dma_start(
                v_sb,
                v_dram[b].rearrange("(kb p) f -> p kb f", p=128)[
                    :, :, h * HD : (h + 1) * HD
                ],
            )
            return qt_sb, kt_sb, v_sb

        bh_pairs = [(b, h) for b in range(B_LOC) for h in range(H_LOC)]
        pending = {}
        pending[bh_pairs[0]] = emit_head_load(*bh_pairs[0])

        y_by_batch = {b: [] for b in range(B_LOC)}
        for bh_i, (b, h) in enumerate(bh_pairs):
            y_tiles = y_by_batch[b]
            if True:
                if bh_i + 1 < len(bh_pairs):
                    pending[bh_pairs[bh_i + 1]] = emit_head_load(*bh_pairs[bh_i + 1])
                qt_sb, kt_sb, v_sb = pending.pop((b, h))
                y_sb = ypool.tile([HD, T], F32R, name="y_sb")
                y_tiles.append(y_sb)

                norm_pairs = []
                for jq in range(NQT - 1, -1, -1):
                    qsl = slice(jq * QTILE, (jq + 1) * QTILE)
                    nkb = 4 * (jq + 1)
                    y_ps = ypsum.tile([HD, QTILE], F32, name="y_ps")
                    l_ps = lpsum.tile([1, QTILE], F32, name="l_ps")
                    for kb in range(nkb):
                        # diagonal blocks only touch q >= qoff within this tile
                        s_diag = kb - 4 * jq
                        qoff = 128 * s_diag if s_diag > 0 else 0
                        qn = QTILE - qoff
                        qsub = slice(jq * QTILE + qoff, (jq + 1) * QTILE)
                        s_ps = spsum.tile([128, QTILE], F32, name="s_ps")
                        nc.tensor.matmul(
                            s_ps[:, qoff:],
                            kt_sb[:, kb * 128 : (kb + 1) * 128],
                            qt_sb[:, qsub],
                            start=True,
                            stop=True,
                        )
                        p_sb = ppool.tile([128, QTILE], F32R, name="p_sb")
                        nc.scalar.activation(
                            p_sb[:, qoff:], s_ps[:, qoff:], AF.Exp, scale=SCALE
                        )
                        if s_diag >= 0:
                            # causal: keep where (q - qoff) - k >= 0 in sub-range
                            nc.gpsimd.affine_select(
                                out=p_sb[:, qoff:],
                                in_=p_sb[:, qoff:],
                                pattern=[[1, qn]],
                                compare_op=ALU.is_ge,
                                fill=0.0,
                                base=0,
                                channel_multiplier=-1,
                            )
                        nc.tensor.matmul(
                            l_ps[:, qoff:],
                            ones_c_sb,
                            p_sb[:, qoff:],
                            start=(kb == 0),
                            stop=(kb == nkb - 1),
                        )
                        nc.tensor.matmul(
                            y_ps[:, qoff:],
                            v_sb[:, kb, :],
                            p_sb[:, qoff:],
                            start=(kb == 0),
                            stop=(kb == nkb - 1),
                        )
                    # evacuate y unnormalized immediately (frees the psum bank),
                    # then normalize in place once 1/l is broadcast
                    nc.vector.tensor_copy(y_sb[:, qsl], y_ps)
                    linv = npool.tile([1, QTILE], F32, name="linv")
                    nc.vector.reciprocal_approx_fast(linv, l_ps)
                    linv_dr = nbounce.tile([1, QTILE], F32, name="linv_dr")
                    nc.sync.dma_start(linv_dr, linv)
                    bc_sb = npool.tile([128, QTILE], F32, name="bc_sb")
                    nc.sync.dma_start(bc_sb, linv_dr.to_broadcast([128, QTILE]))
                    norm_pairs.append((qsl, bc_sb))

                # normalize at head end: broadcasts already in flight, so these
                # don't block the DVE stream mid-pipeline
                for qsl_n, bc_n in norm_pairs:
                    nc.vector.tensor_tensor(
                        y_sb[:, qsl_n], y_sb[:, qsl_n], bc_n, ALU.mult
                    )

            if h != H_LOC - 1:
                continue
            # output projection for this batch: out^T = Wp.T @ y^T
            for jt in range(NQT):
                tsl = slice(jt * QTILE, (jt + 1) * QTILE)
                for co in range(C // 128):
                    o_ps = opsum.tile([128, QTILE], F32, name="o_ps")
                    for h in range(H_LOC):
                        nc.tensor.matmul(
                            o_ps,
                            wp_sb[:, h, co * 128 : (co + 1) * 128],
                            y_tiles[h][:, tsl],
                            start=(h == 0),
                            stop=(h == H_LOC - 1),
                        )
                    o_sb = opool.tile([128, QTILE], F32, name="o_sb")
                    nc.vector.tensor_copy(o_sb, o_ps)
                    nc.sync.dma_start(
                        out_t[b, co * 128 : (co + 1) * 128, tsl], o_sb
                    )


def _get_nc():
    if "nc" not in _CACHED:
        _CACHED["nc"] = build_nc()
    return _CACHED["nc"]


def kernel(x, sin, cos, W_qkv, W_proj):
    x = np.asarray(x, dtype=np.float32)
    sin = np.asarray(sin, dtype=np.float32)
    cos = np.asarray(cos, dtype=np.float32)
    W_qkv = np.asarray(W_qkv, dtype=np.float32)
    W_proj = np.asarray(W_proj, dtype=np.float32)

    sin_t = np.ascontiguousarray(sin[0, 0].T)  # [HD, T]
    cos_t = np.ascontiguousarray(cos[0, 0].T)
    pt = rope_perm_matrix()
    ones_col = np.ones((128, 1), np.float32)
    ones_row = np.ones((1, 128), np.float32)

    in_maps = []
    for g in range(BGROUPS):
        x_tg = np.ascontiguousarray(
            x[g * B_LOC : (g + 1) * B_LOC].transpose(0, 2, 1)
        )  # [B_LOC, C, T]
        for s in range(HSHARDS):
            qcols = W_qkv[:, s * FQK : (s + 1) * FQK]
            kcols = W_qkv[:, C + s * FQK : C + (s + 1) * FQK]
            vcols = W_qkv[:, 2 * C + s * FV : 2 * C + (s + 1) * FV]
            w_qkv_loc = np.ascontiguousarray(
                np.concatenate([qcols, kcols, vcols], axis=1)
            )
            w_proj_loc = np.ascontiguousarray(W_proj[s * FV : (s + 1) * FV, :])
            in_maps.append(
                {
                    "x_t": x_tg,
                    "w_qkv": w_qkv_loc,
                    "w_proj": w_proj_loc,
                    "sin_t": sin_t,
                    "cos_t": cos_t,
                    "pt": pt,
                    "ones_col": ones_col,
                    "ones_row": ones_row,
                }
            )

    trace = bool(int(os.environ.get("KERNEL_TRACE", "0")))
    if trace:
        _install_ntff_hook()
    nc = _get_nc()
    res = run_bass_kernel_spmd(
        nc, in_maps, core_ids=list(range(NCORES)), trace=trace
    )
    _CACHED["last_result"] = res

    out = np.zeros((B, T, C), dtype=np.float32)
    for g in range(BGROUPS):
        acc = np.zeros((B_LOC, C, T), dtype=np.float32)
        for s in range(HSHARDS):
            acc += res.results[g * HSHARDS + s]["out_t"]
        out[g * B_LOC : (g + 1) * B_LOC] = acc.transpose(0, 2, 1)
    return out



# revision 3
# speedup vs baseline: 1.2012x; 1.2012x over previous
"""Trainium2 Bass kernel for nn_MHA_43095701848407.

MHA forward: qkv = x @ W_qkv, RoPE on q/k, causal softmax attention,
y @ W_proj.  B=4, T=2048, C=2048, 16 heads, head_dim=128, fp32 I/O.

Sharding (8 cores): tensor-parallel over heads (4 shards x 4 heads) x
data-parallel over batch (2 groups x 2 batches).  core = group*4 + shard.

v2 design (vs v1 baseline at ~862us):
  - all matmul operands bf16 (fp32 PSUM accumulation): LDWEIGHTS drops
    from 224ns to ~107ns and hides under the 213ns matmul stream, DMA
    bytes halve.  fp32 I/O converted host-side.
  - single fused emission with software-pipelined stages so the
    ACT-heavy attention of batch b overlaps the tensor-heavy qkv of
    batch b+1, and attention of b1 overlaps the projection of b0.
  - q/k/v round-trip DRAM in bf16; attention loads issued from the sync
    queue (v1 used the scalar queue, which delayed the exp stream).
  - softmax 1/l broadcast via gpsimd.partition_broadcast instead of a
    DMA bounce through DRAM.
  - y tiles stay in SBUF (bf16) and feed the projection directly; the
    projection emits per 512-token chunk interleaved with attention.
Host sums the 4 head-shard partial outputs (bf16) per batch and
transposes back.

Self-contained: shapes/sharding hardcoded; inputs full-size numpy arrays.
"""

import math
import os
import sys
import types

import numpy as np
import ml_dtypes

import concourse.bass as bass
import concourse.mybir as mybir
import concourse.tile as tile
from concourse import bacc
from concourse.bass_utils import run_bass_kernel_spmd

F32 = mybir.dt.float32
BF16 = mybir.dt.bfloat16
AF = mybir.ActivationFunctionType
ALU = mybir.AluOpType

# Problem shape (hardcoded per contract)
B, T, C = 4, 2048, 2048
H, HD = 16, 128
NCORES = 8
BGROUPS, HSHARDS = 2, 4  # batch groups x head shards
B_LOC = B // BGROUPS  # 2 batches per core
H_LOC = H // HSHARDS  # 4 heads per core
FQK = H_LOC * HD  # 512 features for q (and for k)
FV = H_LOC * HD  # 512 features for v
F_ALL = 3 * H_LOC * HD  # 1536 qkv features per core
KO = C // 128  # 16 contraction chunks
TSLAB = 512
NSLAB = T // TSLAB  # 4 t-slabs per batch
QTILE = 512
NQT = T // QTILE  # 4 q-tiles
NKB = T // 128  # 16 key blocks
SCALE = 1.0 / math.sqrt(HD)

_CACHED = {}


def _install_ntff_hook():
    """Register the axon NTFF profile hook (container's antenv lacks it)."""
    if "antenv.axon_hooks" in sys.modules:
        return
    try:
        mod = types.ModuleType("antenv.axon_hooks")
        holder = [None]
        mod.set_axon_ntff_profile_hook = lambda h: holder.__setitem__(0, h)
        mod.get_axon_ntff_profile_hook = lambda: holder[0]
        sys.modules["antenv.axon_hooks"] = mod
        import antenv

        antenv.axon_hooks = mod
        if "/root/.axon_site" not in sys.path:
            sys.path.insert(0, "/root/.axon_site")
        from trn_agent_boot.trn_boot import _ntff_profile_via_ctypes

        mod.set_axon_ntff_profile_hook(
            _ntff_profile_via_ctypes("/opt/axon/libaxon_pjrt.so")
        )
    except Exception:
        sys.modules.pop("antenv.axon_hooks", None)


def rope_perm_matrix():
    """lhsT for the rotate-half matmul: rot^T = PT.T @ q^T.
    rot[2i] = -q[2i+1], rot[2i+1] = q[2i]."""
    pt = np.zeros((HD, HD), dtype=np.float32)
    for i in range(HD // 2):
        pt[2 * i + 1, 2 * i] = -1.0
        pt[2 * i, 2 * i + 1] = 1.0
    return pt


def build_nc():
    nc = bacc.Bacc("TRN2", target_bir_lowering=False, debug=False)

    x_t = nc.dram_tensor("x_t", [B_LOC, C, T], BF16, kind="ExternalInput").ap()
    w_qkv = nc.dram_tensor("w_qkv", [C, F_ALL], BF16, kind="ExternalInput").ap()
    w_proj = nc.dram_tensor("w_proj", [FV, C], BF16, kind="ExternalInput").ap()
    sin_t = nc.dram_tensor("sin_t", [HD, T], BF16, kind="ExternalInput").ap()
    cos_t = nc.dram_tensor("cos_t", [HD, T], BF16, kind="ExternalInput").ap()
    pt = nc.dram_tensor("pt", [HD, HD], BF16, kind="ExternalInput").ap()
    ones_col = nc.dram_tensor("ones_col", [128, 1], BF16, kind="ExternalInput").ap()
    out_t = nc.dram_tensor("out_t", [B_LOC, C, T], BF16, kind="ExternalOutput").ap()

    with tile.TileContext(nc) as tc:
        with nc.allow_low_precision(reason="bf16 matmul pipeline by design"):
            _emit(nc, tc, x_t, w_qkv, w_proj, sin_t, cos_t, pt, ones_col, out_t)
    nc.compile()
    return nc


def _emit(nc, tc, x_t, w_qkv, w_proj, sin_t, cos_t, pt, ones_col, out_t):
    with (
        tc.tile_pool(name="dram", bufs=1, space="DRAM") as dram_pool,
        tc.tile_pool(name="consts", bufs=1) as consts,
        tc.tile_pool(name="wq", bufs=1) as wqpool,
        tc.tile_pool(name="wp", bufs=1) as wppool,
        tc.tile_pool(name="sc", bufs=1) as scpool,
        tc.tile_pool(name="xpool", bufs=2) as xpool,
        tc.tile_pool(name="rope", bufs=2) as ropepool,
        tc.tile_pool(name="qkvload", bufs=2) as qkvload,
        tc.tile_pool(name="vload", bufs=2) as vload,
        tc.tile_pool(name="ppool", bufs=6) as ppool,
        tc.tile_pool(name="ypool", bufs=2 * H_LOC) as ypool,
        tc.tile_pool(name="npool", bufs=2) as npool,
        tc.tile_pool(name="opool", bufs=3) as opool,
        tc.tile_pool(name="accum", bufs=3, space="PSUM") as accum,
        tc.tile_pool(name="spsum", bufs=2, space="PSUM") as spsum,
        tc.tile_pool(name="ypsum", bufs=2, space="PSUM") as ypsum,
        tc.tile_pool(name="lpsum", bufs=1, space="PSUM") as lpsum,
    ):
        qk_dram = [
            dram_pool.tile([2 * FQK, T], BF16, name=f"qk_dram{b}")
            for b in range(B_LOC)
        ]
        v_dram = [
            dram_pool.tile([T, FV], BF16, name=f"v_dram{b}") for b in range(B_LOC)
        ]

        pt_sb = consts.tile([HD, HD], BF16)
        nc.sync.dma_start(pt_sb, pt)
        ones_c_sb = consts.tile([128, 1], BF16)
        nc.sync.dma_start(ones_c_sb, ones_col)

        w_sb = wqpool.tile([128, KO, F_ALL], BF16)
        w_src = w_qkv.rearrange("(ko p) f -> p ko f", p=128)
        for ko in range(KO):
            nc.sync.dma_start(w_sb[:, ko, :], w_src[:, ko, :])
        wp_sb = wppool.tile([128, H_LOC, C], BF16)
        nc.sync.dma_start(wp_sb, w_proj.rearrange("(fo p) c -> p fo c", p=128))
        sin_sb = scpool.tile([HD, T], BF16)
        nc.sync.dma_start(sin_sb, sin_t)
        cos_sb = scpool.tile([HD, T], BF16)
        nc.sync.dma_start(cos_sb, cos_t)

        # ---------- qkv phase: one t-slab of 512 tokens ----------
        def emit_qkv_slab(b, js, first=False):
            tsl = slice(js * TSLAB, (js + 1) * TSLAB)
            x3 = x_t[b].rearrange("(ko p) t -> p ko t", p=128)
            x_sb = xpool.tile([128, KO, TSLAB], BF16, name="x_sb")
            if first:
                # split by ko so the first matmuls start after ~1/16 load
                for ko in range(KO):
                    nc.sync.dma_start(x_sb[:, ko, :], x3[:, ko, tsl])
            else:
                nc.sync.dma_start(x_sb, x3[:, :, tsl])

            # q^T, k^T feature chunks (one head each) with RoPE.
            # rot-matmul+combine for chunk f is emitted one chunk late so
            # the tensor queue never waits on the raw-copy cast.
            pending = None  # (raw_bf, f)

            def flush_rope():
                raw_bf, f = pending
                rot_ps = accum.tile([128, TSLAB], F32, name="rot_ps", tag="acc")
                nc.tensor.matmul(rot_ps, pt_sb, raw_bf, start=True, stop=True)
                t1 = ropepool.tile([128, TSLAB], BF16, name="t1")
                nc.gpsimd.tensor_tensor(t1, raw_bf, cos_sb[:, tsl], ALU.mult)
                t2 = ropepool.tile([128, TSLAB], BF16, name="t2")
                nc.vector.tensor_tensor(t2, rot_ps, sin_sb[:, tsl], ALU.mult)
                roped = ropepool.tile([128, TSLAB], BF16, name="roped")
                nc.vector.tensor_tensor(roped, t1, t2, ALU.add)
                nc.sync.dma_start(qk_dram[b][f * 128 : (f + 1) * 128, tsl], roped)

            for f in range(2 * H_LOC):
                ps = accum.tile([128, TSLAB], F32, name="qk_ps", tag="acc")
                for ko in range(KO):
                    nc.tensor.matmul(
                        ps,
                        w_sb[:, ko, f * 128 : (f + 1) * 128],
                        x_sb[:, ko, :],
                        start=(ko == 0),
                        stop=(ko == KO - 1),
                    )
                raw_bf = ropepool.tile([128, TSLAB], BF16, name="raw_bf", bufs=3)
                nc.vector.tensor_copy(raw_bf, ps)
                if pending is not None:
                    flush_rope()
                pending = (raw_bf, f)

            # v in natural [t, f] layout
            for tb in range(TSLAB // 128):
                vps = accum.tile([128, FV], F32, name="v_ps", tag="acc")
                for ko in range(KO):
                    nc.tensor.matmul(
                        vps,
                        x_sb[:, ko, tb * 128 : (tb + 1) * 128],
                        w_sb[:, ko, 2 * FQK : 2 * FQK + FV],
                        start=(ko == 0),
                        stop=(ko == KO - 1),
                    )
                if pending is not None:
                    flush_rope()
                    pending = None
                v_bf = ropepool.tile([128, FV], BF16, name="v_bf", bufs=3)
                nc.vector.tensor_copy(v_bf, vps)
                r0 = js * TSLAB + tb * 128
                nc.sync.dma_start(v_dram[b][r0 : r0 + 128, :], v_bf)

        # ---------- attention ----------
        head_loads = {}
        vpair_loads = {}

        def emit_head_load(b, h):
            qt_sb = qkvload.tile([HD, T], BF16, name="qt_sb")
            nc.sync.dma_start(qt_sb, qk_dram[b][h * HD : (h + 1) * HD, :])
            kt_sb = qkvload.tile([HD, T], BF16, name="kt_sb")
            nc.sync.dma_start(
                kt_sb, qk_dram[b][FQK + h * HD : FQK + (h + 1) * HD, :]
            )
            head_loads[(b, h)] = (qt_sb, kt_sb)
            if h % 2 == 0:
                v_sb = vload.tile([128, NKB, 2 * HD], BF16, name="v_sb")
                nc.sync.dma_start(
                    v_sb,
                    v_dram[b].rearrange("(kb p) f -> p kb f", p=128)[
                        :, :, h * HD : (h + 2) * HD
                    ],
                )
                vpair_loads[(b, h // 2)] = v_sb

        bh_order = [(b, h) for b in range(B_LOC) for h in range(H_LOC)]

        def emit_attn_head(b, h):
            # prefetch next head's q/k/v
            i = bh_order.index((b, h))
            if i + 1 < len(bh_order):
                nb, nh = bh_order[i + 1]
                if (nb, nh) not in head_loads:
                    emit_head_load(nb, nh)
            qt_sb, kt_sb = head_loads.pop((b, h))
            v_sb = vpair_loads[(b, h // 2)]
            vc = (h % 2) * HD
            y_sb = ypool.tile([HD, T], BF16, name="y_sb")
            for jq in range(NQT):
                qsl = slice(jq * QTILE, (jq + 1) * QTILE)
                nkb = 4 * (jq + 1)
                y_ps = ypsum.tile([HD, QTILE], F32, name="y_ps")
                l_ps = lpsum.tile([1, QTILE], F32, name="l_ps")
                for kb in range(nkb):
                    # diagonal blocks only touch q >= qoff within this tile
                    s_diag = kb - 4 * jq
                    qoff = 128 * s_diag if s_diag > 0 else 0
                    qn = QTILE - qoff
                    qsub = slice(jq * QTILE + qoff, (jq + 1) * QTILE)
                    s_ps = spsum.tile([128, QTILE], F32, name="s_ps")
                    nc.tensor.matmul(
                        s_ps[:, qoff:],
                        kt_sb[:, kb * 128 : (kb + 1) * 128],
                        qt_sb[:, qsub],
                        start=True,
                        stop=True,
                    )
                    p_bf = ppool.tile([128, QTILE], BF16, name="p_bf")
                    nc.scalar.activation(
                        p_bf[:, qoff:], s_ps[:, qoff:], AF.Exp, scale=SCALE
                    )
                    if s_diag >= 0:
                        # causal: keep where (q - qoff) - k >= 0 in sub-range
                        nc.gpsimd.affine_select(
                            out=p_bf[:, qoff:],
                            in_=p_bf[:, qoff:],
                            pattern=[[1, qn]],
                            compare_op=ALU.is_ge,
                            fill=0.0,
                            base=0,
                            channel_multiplier=-1,
                        )
                    nc.tensor.matmul(
                        l_ps[:, qoff:],
                        ones_c_sb,
                        p_bf[:, qoff:],
                        start=(kb == 0),
                        stop=(kb == nkb - 1),
                    )
                    nc.tensor.matmul(
                        y_ps[:, qoff:],
                        v_sb[:, kb, vc : vc + HD],
                        p_bf[:, qoff:],
                        start=(kb == 0),
                        stop=(kb == nkb - 1),
                    )
                linv = npool.tile([1, QTILE], F32, name="linv")
                nc.vector.reciprocal_approx_fast(linv, l_ps)
                bc = npool.tile([128, QTILE], F32, name="bc")
                nc.gpsimd.partition_broadcast(bc, linv)
                # fused normalize + evacuation: y = y_ps * (1/l)
                nc.vector.tensor_tensor(y_sb[:, qsl], y_ps, bc, ALU.mult)
            return y_sb

        # ---------- output projection, one 512-token chunk ----------
        def emit_proj_chunk(b, jt, y_tiles):
            tsl = slice(jt * QTILE, (jt + 1) * QTILE)
            for co in range(C // 128):
                o_ps = accum.tile([128, QTILE], F32, name="o_ps", tag="acc")
                for h in range(H_LOC):
                    nc.tensor.matmul(
                        o_ps,
                        wp_sb[:, h, co * 128 : (co + 1) * 128],
                        y_tiles[h][:, tsl],
                        start=(h == 0),
                        stop=(h == H_LOC - 1),
                    )
                o_bf = opool.tile([128, QTILE], BF16, name="o_bf")
                nc.vector.tensor_copy(o_bf, o_ps)
                nc.sync.dma_start(out_t[b, co * 128 : (co + 1) * 128, tsl], o_bf)

        # ---------- fused schedule ----------
        y_by_batch = {0: [], 1: []}
        # stage 0: qkv(b0)
        for js in range(NSLAB):
            emit_qkv_slab(0, js, first=(js == 0))
        # stage 1: qkv(b1) slabs interleaved with attn(b0) heads
        for i in range(NSLAB):
            emit_qkv_slab(1, i)
            if i == 0:
                emit_head_load(0, 0)
            y_by_batch[0].append(emit_attn_head(0, i))
        # stage 2: attn(b1) heads interleaved with proj(b0) chunks
        for i in range(H_LOC):
            y_by_batch[1].append(emit_attn_head(1, i))
            emit_proj_chunk(0, i, y_by_batch[0])
        # stage 3: proj(b1)
        for jt in range(NQT):
            emit_proj_chunk(1, jt, y_by_batch[1])


def _get_nc():
    if "nc" not in _CACHED:
        _CACHED["nc"] = build_nc()
    return _CACHED["nc"]


def kernel(x, sin, cos, W_qkv, W_proj):
    x = np.asarray(x, dtype=np.float32)
    sin = np.asarray(sin, dtype=np.float32)
    cos = np.asarray(cos, dtype=np.float32)
    W_qkv = np.asarray(W_qkv, dtype=np.float32)
    W_proj = np.asarray(W_proj, dtype=np.float32)

    bf = ml_dtypes.bfloat16
    sin_t = np.ascontiguousarray(sin[0, 0].T).astype(bf)  # [HD, T]
    cos_t = np.ascontiguousarray(cos[0, 0].T).astype(bf)
    pt = rope_perm_matrix().astype(bf)
    ones_col = np.ones((128, 1), bf)

    in_maps = []
    for g in range(BGROUPS):
        x_tg = np.ascontiguousarray(
            x[g * B_LOC : (g + 1) * B_LOC].transpose(0, 2, 1)
        ).astype(bf)  # [B_LOC, C, T]
        for s in range(HSHARDS):
            qcols = W_qkv[:, s * FQK : (s + 1) * FQK]
            kcols = W_qkv[:, C + s * FQK : C + (s + 1) * FQK]
            vcols = W_qkv[:, 2 * C + s * FV : 2 * C + (s + 1) * FV]
            w_qkv_loc = np.ascontiguousarray(
                np.concatenate([qcols, kcols, vcols], axis=1)
            ).astype(bf)
            w_proj_loc = np.ascontiguousarray(
                W_proj[s * FV : (s + 1) * FV, :]
            ).astype(bf)
            in_maps.append(
                {
                    "x_t": x_tg,
                    "w_qkv": w_qkv_loc,
                    "w_proj": w_proj_loc,
                    "sin_t": sin_t,
                    "cos_t": cos_t,
                    "pt": pt,
                    "ones_col": ones_col,
                }
            )

    trace = bool(int(os.environ.get("KERNEL_TRACE", "0")))
    if trace:
        _install_ntff_hook()
    nc = _get_nc()
    res = run_bass_kernel_spmd(
        nc, in_maps, core_ids=list(range(NCORES)), trace=trace
    )
    _CACHED["last_result"] = res

    out = np.zeros((B, T, C), dtype=np.float32)
    for g in range(BGROUPS):
        acc = np.zeros((B_LOC, C, T), dtype=np.float32)
        for s in range(HSHARDS):
            acc += np.asarray(res.results[g * HSHARDS + s]["out_t"], dtype=np.float32)
        out[g * B_LOC : (g + 1) * B_LOC] = acc.transpose(0, 2, 1)
    return out


# revision 4
# speedup vs baseline: 1.2973x; 1.0800x over previous
"""Trainium2 Bass kernel for nn_MHA_43095701848407.

MHA forward: qkv = x @ W_qkv, RoPE on q/k, causal softmax attention,
y @ W_proj.  B=4, T=2048, C=2048, 16 heads, head_dim=128, fp32 I/O.

Sharding (8 cores): tensor-parallel over heads (4 shards x 4 heads) x
data-parallel over batch (2 groups x 2 batches).  core = group*4 + shard.

v2 design (vs v1 baseline at ~862us):
  - all matmul operands bf16 (fp32 PSUM accumulation): LDWEIGHTS drops
    from 224ns to ~107ns and hides under the 213ns matmul stream, DMA
    bytes halve.  fp32 I/O converted host-side.
  - single fused emission with software-pipelined stages so the
    ACT-heavy attention of batch b overlaps the tensor-heavy qkv of
    batch b+1, and attention of b1 overlaps the projection of b0.
  - q/k/v round-trip DRAM in bf16; attention loads issued from the sync
    queue (v1 used the scalar queue, which delayed the exp stream).
  - softmax 1/l broadcast via gpsimd.partition_broadcast instead of a
    DMA bounce through DRAM.
  - y tiles stay in SBUF (bf16) and feed the projection directly; the
    projection emits per 512-token chunk interleaved with attention.
Host sums the 4 head-shard partial outputs (bf16) per batch and
transposes back.

Self-contained: shapes/sharding hardcoded; inputs full-size numpy arrays.
"""

import math
import os
import sys
import types

import numpy as np
import ml_dtypes

import concourse.bass as bass
import concourse.mybir as mybir
import concourse.tile as tile
from concourse import bacc
from concourse.bass_utils import run_bass_kernel_spmd

F32 = mybir.dt.float32
BF16 = mybir.dt.bfloat16
AF = mybir.ActivationFunctionType
ALU = mybir.AluOpType

# Problem shape (hardcoded per contract)
B, T, C = 4, 2048, 2048
H, HD = 16, 128
NCORES = 8
BGROUPS, HSHARDS = 2, 4  # batch groups x head shards
B_LOC = B // BGROUPS  # 2 batches per core
H_LOC = H // HSHARDS  # 4 heads per core
FQK = H_LOC * HD  # 512 features for q (and for k)
FV = H_LOC * HD  # 512 features for v
F_ALL = 3 * H_LOC * HD  # 1536 qkv features per core
KO = C // 128  # 16 contraction chunks
TSLAB = 512
NSLAB = T // TSLAB  # 4 t-slabs per batch
QTILE = 512
NQT = T // QTILE  # 4 q-tiles
NKB = T // 128  # 16 key blocks
SCALE = 1.0 / math.sqrt(HD)

_CACHED = {}


def _install_ntff_hook():
    """Register the axon NTFF profile hook (container's antenv lacks it)."""
    if "antenv.axon_hooks" in sys.modules:
        return
    try:
        mod = types.ModuleType("antenv.axon_hooks")
        holder = [None]
        mod.set_axon_ntff_profile_hook = lambda h: holder.__setitem__(0, h)
        mod.get_axon_ntff_profile_hook = lambda: holder[0]
        sys.modules["antenv.axon_hooks"] = mod
        import antenv

        antenv.axon_hooks = mod
        if "/root/.axon_site" not in sys.path:
            sys.path.insert(0, "/root/.axon_site")
        from trn_agent_boot.trn_boot import _ntff_profile_via_ctypes

        mod.set_axon_ntff_profile_hook(
            _ntff_profile_via_ctypes("/opt/axon/libaxon_pjrt.so")
        )
    except Exception:
        sys.modules.pop("antenv.axon_hooks", None)


def rope_perm_matrix():
    """lhsT for the rotate-half matmul: rot^T = PT.T @ q^T.
    rot[2i] = -q[2i+1], rot[2i+1] = q[2i]."""
    pt = np.zeros((HD, HD), dtype=np.float32)
    for i in range(HD // 2):
        pt[2 * i + 1, 2 * i] = -1.0
        pt[2 * i, 2 * i + 1] = 1.0
    return pt


def build_nc():
    nc = bacc.Bacc("TRN2", target_bir_lowering=False, debug=False)

    x_t = nc.dram_tensor("x_t", [B_LOC, C, T], BF16, kind="ExternalInput").ap()
    w_qkv = nc.dram_tensor("w_qkv", [C, F_ALL], BF16, kind="ExternalInput").ap()
    w_proj = nc.dram_tensor("w_proj", [FV, C], BF16, kind="ExternalInput").ap()
    sin_t = nc.dram_tensor("sin_t", [HD, T], BF16, kind="ExternalInput").ap()
    cos_t = nc.dram_tensor("cos_t", [HD, T], BF16, kind="ExternalInput").ap()
    pt = nc.dram_tensor("pt", [HD, HD], BF16, kind="ExternalInput").ap()
    ones_col = nc.dram_tensor("ones_col", [128, 1], BF16, kind="ExternalInput").ap()
    out_t = nc.dram_tensor("out_t", [B_LOC, C, T], BF16, kind="ExternalOutput").ap()

    with tile.TileContext(nc) as tc:
        with nc.allow_low_precision(reason="bf16 matmul pipeline by design"):
            _emit(nc, tc, x_t, w_qkv, w_proj, sin_t, cos_t, pt, ones_col, out_t)
    nc.compile()
    return nc


def _emit(nc, tc, x_t, w_qkv, w_proj, sin_t, cos_t, pt, ones_col, out_t):
    with (
        tc.tile_pool(name="dram", bufs=1, space="DRAM") as dram_pool,
        tc.tile_pool(name="consts", bufs=1) as consts,
        tc.tile_pool(name="wq", bufs=1) as wqpool,
        tc.tile_pool(name="wp", bufs=1) as wppool,
        tc.tile_pool(name="sc", bufs=1) as scpool,
        tc.tile_pool(name="xpool", bufs=2) as xpool,
        tc.tile_pool(name="rope", bufs=2) as ropepool,
        tc.tile_pool(name="qkvload", bufs=2) as qkvload,
        tc.tile_pool(name="vload", bufs=2) as vload,
        tc.tile_pool(name="ppool", bufs=6) as ppool,
        tc.tile_pool(name="ypool", bufs=2 * H_LOC) as ypool,
        tc.tile_pool(name="npool", bufs=2) as npool,
        tc.tile_pool(name="opool", bufs=3) as opool,
        tc.tile_pool(name="accum", bufs=3, space="PSUM") as accum,
        tc.tile_pool(name="spsum", bufs=2, space="PSUM") as spsum,
        tc.tile_pool(name="ypsum", bufs=2, space="PSUM") as ypsum,
        tc.tile_pool(name="lpsum", bufs=1, space="PSUM") as lpsum,
    ):
        qk_dram = [
            dram_pool.tile([2 * FQK, T], BF16, name=f"qk_dram{b}")
            for b in range(B_LOC)
        ]
        v_dram = [
            dram_pool.tile([T, FV], BF16, name=f"v_dram{b}") for b in range(B_LOC)
        ]

        pt_sb = consts.tile([HD, HD], BF16)
        nc.sync.dma_start(pt_sb, pt)
        ones_c_sb = consts.tile([128, 1], BF16)
        nc.sync.dma_start(ones_c_sb, ones_col)

        sin_sb = scpool.tile([HD, T], BF16)
        nc.sync.dma_start(sin_sb, sin_t)
        cos_sb = scpool.tile([HD, T], BF16)
        nc.sync.dma_start(cos_sb, cos_t)
        # w chunks are interleaved with the first x slab's chunks below so
        # the first matmul starts after ~1/16 of each load; wp loads at the
        # end of stage 1 (first needed by proj in stage 2).
        w_sb = wqpool.tile([128, KO, F_ALL], BF16)
        w_src = w_qkv.rearrange("(ko p) f -> p ko f", p=128)
        wp_sb = wppool.tile([128, H_LOC, C], BF16)

        # ---------- qkv phase: one t-slab of 512 tokens ----------
        def emit_qkv_slab(b, js, first=False):
            tsl = slice(js * TSLAB, (js + 1) * TSLAB)
            x3 = x_t[b].rearrange("(ko p) t -> p ko t", p=128)
            x_sb = xpool.tile([128, KO, TSLAB], BF16, name="x_sb")
            if first:
                # interleave weight and x chunk loads so the ko=0 matmul
                # starts after ~1/16 of each tensor has landed
                for ko in range(KO):
                    nc.sync.dma_start(w_sb[:, ko, :], w_src[:, ko, :])
                    nc.sync.dma_start(x_sb[:, ko, :], x3[:, ko, tsl])
            else:
                nc.sync.dma_start(x_sb, x3[:, :, tsl])

            # q^T, k^T feature chunks (one head each) with RoPE.
            # rot-matmul+combine for chunk f is emitted one chunk late so
            # the tensor queue never waits on the raw-copy cast.
            pending = None  # (raw_bf, f)

            def flush_rope():
                raw_bf, f = pending
                rot_ps = accum.tile([128, TSLAB], F32, name="rot_ps", tag="acc")
                nc.tensor.matmul(rot_ps, pt_sb, raw_bf, start=True, stop=True)
                t1 = ropepool.tile([128, TSLAB], BF16, name="t1")
                nc.vector.tensor_tensor(t1, raw_bf, cos_sb[:, tsl], ALU.mult)
                t2 = ropepool.tile([128, TSLAB], BF16, name="t2")
                nc.vector.tensor_tensor(t2, rot_ps, sin_sb[:, tsl], ALU.mult)
                roped = ropepool.tile([128, TSLAB], BF16, name="roped")
                nc.vector.tensor_tensor(roped, t1, t2, ALU.add)
                nc.sync.dma_start(qk_dram[b][f * 128 : (f + 1) * 128, tsl], roped)

            for f in range(2 * H_LOC):
                ps = accum.tile([128, TSLAB], F32, name="qk_ps", tag="acc")
                for ko in range(KO):
                    nc.tensor.matmul(
                        ps,
                        w_sb[:, ko, f * 128 : (f + 1) * 128],
                        x_sb[:, ko, :],
                        start=(ko == 0),
                        stop=(ko == KO - 1),
                    )
                raw_bf = ropepool.tile([128, TSLAB], BF16, name="raw_bf", bufs=3)
                nc.vector.tensor_copy(raw_bf, ps)
                if pending is not None:
                    flush_rope()
                pending = (raw_bf, f)

            # v in natural [t, f] layout
            for tb in range(TSLAB // 128):
                vps = accum.tile([128, FV], F32, name="v_ps", tag="acc")
                for ko in range(KO):
                    nc.tensor.matmul(
                        vps,
                        x_sb[:, ko, tb * 128 : (tb + 1) * 128],
                        w_sb[:, ko, 2 * FQK : 2 * FQK + FV],
                        start=(ko == 0),
                        stop=(ko == KO - 1),
                    )
                if pending is not None:
                    flush_rope()
                    pending = None
                v_bf = ropepool.tile([128, FV], BF16, name="v_bf", bufs=3)
                nc.vector.tensor_copy(v_bf, vps)
                r0 = js * TSLAB + tb * 128
                nc.sync.dma_start(v_dram[b][r0 : r0 + 128, :], v_bf)

        # ---------- attention ----------
        head_loads = {}
        vpair_loads = {}

        def emit_head_load(b, h):
            qt_sb = qkvload.tile([HD, T], BF16, name="qt_sb")
            nc.sync.dma_start(qt_sb, qk_dram[b][h * HD : (h + 1) * HD, :])
            kt_sb = qkvload.tile([HD, T], BF16, name="kt_sb")
            nc.sync.dma_start(
                kt_sb, qk_dram[b][FQK + h * HD : FQK + (h + 1) * HD, :]
            )
            head_loads[(b, h)] = (qt_sb, kt_sb)
            if h % 2 == 0:
                v_sb = vload.tile([128, NKB, 2 * HD], BF16, name="v_sb")
                nc.sync.dma_start(
                    v_sb,
                    v_dram[b].rearrange("(kb p) f -> p kb f", p=128)[
                        :, :, h * HD : (h + 2) * HD
                    ],
                )
                vpair_loads[(b, h // 2)] = v_sb

        bh_order = [(b, h) for b in range(B_LOC) for h in range(H_LOC)]

        def emit_attn_head(b, h):
            # prefetch next head's q/k/v
            i = bh_order.index((b, h))
            if i + 1 < len(bh_order):
                nb, nh = bh_order[i + 1]
                if (nb, nh) not in head_loads:
                    emit_head_load(nb, nh)
            qt_sb, kt_sb = head_loads.pop((b, h))
            v_sb = vpair_loads[(b, h // 2)]
            vc = (h % 2) * HD
            y_sb = ypool.tile([HD, T], BF16, name="y_sb")
            norm_pairs = []
            for jq in range(NQT):
                qsl = slice(jq * QTILE, (jq + 1) * QTILE)
                nkb = 4 * (jq + 1)
                y_ps = ypsum.tile([HD, QTILE], F32, name="y_ps")
                l_ps = lpsum.tile([1, QTILE], F32, name="l_ps")

                # software-pipelined blocks: the score matmul for block kb+1
                # is emitted before the l/y matmuls of block kb, so the
                # tensor queue never waits head-of-line on the exp output.
                def emit_score(kb):
                    s_diag = kb - 4 * jq
                    qoff = 128 * s_diag if s_diag > 0 else 0
                    qn = QTILE - qoff
                    qsub = slice(jq * QTILE + qoff, (jq + 1) * QTILE)
                    s_ps = spsum.tile([128, QTILE], F32, name="s_ps")
                    nc.tensor.matmul(
                        s_ps[:, qoff:],
                        kt_sb[:, kb * 128 : (kb + 1) * 128],
                        qt_sb[:, qsub],
                        start=True,
                        stop=True,
                    )
                    p_bf = ppool.tile([128, QTILE], BF16, name="p_bf")
                    nc.scalar.activation(
                        p_bf[:, qoff:], s_ps[:, qoff:], AF.Exp, scale=SCALE
                    )
                    if s_diag >= 0:
                        # causal: keep where (q - qoff) - k >= 0 in sub-range
                        nc.gpsimd.affine_select(
                            out=p_bf[:, qoff:],
                            in_=p_bf[:, qoff:],
                            pattern=[[1, qn]],
                            compare_op=ALU.is_ge,
                            fill=0.0,
                            base=0,
                            channel_multiplier=-1,
                        )
                    return p_bf, qoff

                def emit_ly(kb, p_bf, qoff):
                    nc.tensor.matmul(
                        l_ps[:, qoff:],
                        ones_c_sb,
                        p_bf[:, qoff:],
                        start=(kb == 0),
                        stop=(kb == nkb - 1),
                    )
                    nc.tensor.matmul(
                        y_ps[:, qoff:],
                        v_sb[:, kb, vc : vc + HD],
                        p_bf[:, qoff:],
                        start=(kb == 0),
                        stop=(kb == nkb - 1),
                    )

                prev = emit_score(0)
                for kb in range(1, nkb):
                    cur = emit_score(kb)
                    emit_ly(kb - 1, *prev)
                    prev = cur
                emit_ly(nkb - 1, *prev)

                linv = npool.tile([1, QTILE], F32, name="linv")
                nc.vector.reciprocal_approx_fast(linv, l_ps)
                linv_bf = npool.tile([1, QTILE], BF16, name="linv_bf")
                nc.vector.tensor_copy(linv_bf, linv)
                bc = npool.tile([128, QTILE], BF16, name="bc", bufs=4)
                nc.gpsimd.partition_broadcast(bc, linv_bf)
                # evacuate unnormalized (frees the psum bank); normalize at
                # head end once the broadcast is surely done, so the DVE
                # queue never waits head-of-line on the recip/bc chain.
                nc.vector.tensor_copy(y_sb[:, qsl], y_ps)
                norm_pairs.append((qsl, bc))
            for qsl_n, bc_n in norm_pairs:
                nc.vector.tensor_tensor(
                    y_sb[:, qsl_n], y_sb[:, qsl_n], bc_n, ALU.mult
                )
            return y_sb

        # ---------- output projection, one 512-token chunk ----------
        def emit_proj_chunk(b, jt, y_tiles):
            tsl = slice(jt * QTILE, (jt + 1) * QTILE)
            for co in range(C // 128):
                o_ps = accum.tile([128, QTILE], F32, name="o_ps", tag="acc")
                for h in range(H_LOC):
                    nc.tensor.matmul(
                        o_ps,
                        wp_sb[:, h, co * 128 : (co + 1) * 128],
                        y_tiles[h][:, tsl],
                        start=(h == 0),
                        stop=(h == H_LOC - 1),
                    )
                o_bf = opool.tile([128, QTILE], BF16, name="o_bf")
                nc.vector.tensor_copy(o_bf, o_ps)
                nc.sync.dma_start(out_t[b, co * 128 : (co + 1) * 128, tsl], o_bf)

        # ---------- fused schedule ----------
        y_by_batch = {0: [], 1: []}
        # stage 0: qkv(b0)
        for js in range(NSLAB):
            emit_qkv_slab(0, js, first=(js == 0))
        # stage 1: qkv(b1) slabs interleaved with attn(b0) heads
        for i in range(NSLAB):
            emit_qkv_slab(1, i)
            if i == 0:
                emit_head_load(0, 0)
            y_by_batch[0].append(emit_attn_head(0, i))
        nc.sync.dma_start(wp_sb, w_proj.rearrange("(fo p) c -> p fo c", p=128))
        # stage 2: attn(b1) heads interleaved with proj(b0) chunks
        for i in range(H_LOC):
            y_by_batch[1].append(emit_attn_head(1, i))
            emit_proj_chunk(0, i, y_by_batch[0])
        # stage 3: proj(b1)
        for jt in range(NQT):
            emit_proj_chunk(1, jt, y_by_batch[1])


def _get_nc():
    if "nc" not in _CACHED:
        _CACHED["nc"] = build_nc()
    return _CACHED["nc"]


def kernel(x, sin, cos, W_qkv, W_proj):
    x = np.asarray(x, dtype=np.float32)
    sin = np.asarray(sin, dtype=np.float32)
    cos = np.asarray(cos, dtype=np.float32)
    W_qkv = np.asarray(W_qkv, dtype=np.float32)
    W_proj = np.asarray(W_proj, dtype=np.float32)

    bf = ml_dtypes.bfloat16
    sin_t = np.ascontiguousarray(sin[0, 0].T).astype(bf)  # [HD, T]
    cos_t = np.ascontiguousarray(cos[0, 0].T).astype(bf)
    pt = rope_perm_matrix().astype(bf)
    ones_col = np.ones((128, 1), bf)

    in_maps = []
    for g in range(BGROUPS):
        x_tg = np.ascontiguousarray(
            x[g * B_LOC : (g + 1) * B_LOC].transpose(0, 2, 1)
        ).astype(bf)  # [B_LOC, C, T]
        for s in range(HSHARDS):
            qcols = W_qkv[:, s * FQK : (s + 1) * FQK]
            kcols = W_qkv[:, C + s * FQK : C + (s + 1) * FQK]
            vcols = W_qkv[:, 2 * C + s * FV : 2 * C + (s + 1) * FV]
            w_qkv_loc = np.ascontiguousarray(
                np.concatenate([qcols, kcols, vcols], axis=1)
            ).astype(bf)
            w_proj_loc = np.ascontiguousarray(
                W_proj[s * FV : (s + 1) * FV, :]
            ).astype(bf)
            in_maps.append(
                {
                    "x_t": x_tg,
                    "w_qkv": w_qkv_loc,
                    "w_proj": w_proj_loc,
                    "sin_t": sin_t,
                    "cos_t": cos_t,
                    "pt": pt,
                    "ones_col": ones_col,
                }
            )

    trace = bool(int(os.environ.get("KERNEL_TRACE", "0")))
    if trace:
        _install_ntff_hook()
    nc = _get_nc()
    res = run_bass_kernel_spmd(
        nc, in_maps, core_ids=list(range(NCORES)), trace=trace
    )
    _CACHED["last_result"] = res

    out = np.zeros((B, T, C), dtype=np.float32)
    for g in range(BGROUPS):
        acc = np.zeros((B_LOC, C, T), dtype=np.float32)
        for s in range(HSHARDS):
            acc += np.asarray(res.results[g * HSHARDS + s]["out_t"], dtype=np.float32)
        out[g * B_LOC : (g + 1) * B_LOC] = acc.transpose(0, 2, 1)
    return out


# revision 5
# speedup vs baseline: 1.3058x; 1.0066x over previous
"""Trainium2 Bass kernel for nn_MHA_43095701848407.

MHA forward: qkv = x @ W_qkv, RoPE on q/k, causal softmax attention,
y @ W_proj.  B=4, T=2048, C=2048, 16 heads, head_dim=128, fp32 I/O.

Sharding (8 cores): tensor-parallel over heads (4 shards x 4 heads) x
data-parallel over batch (2 groups x 2 batches).  core = group*4 + shard.

v2 design (vs v1 baseline at ~862us):
  - all matmul operands bf16 (fp32 PSUM accumulation): LDWEIGHTS drops
    from 224ns to ~107ns and hides under the 213ns matmul stream, DMA
    bytes halve.  fp32 I/O converted host-side.
  - single fused emission with software-pipelined stages so the
    ACT-heavy attention of batch b overlaps the tensor-heavy qkv of
    batch b+1, and attention of b1 overlaps the projection of b0.
  - q/k/v round-trip DRAM in bf16; attention loads issued from the sync
    queue (v1 used the scalar queue, which delayed the exp stream).
  - softmax 1/l broadcast via gpsimd.partition_broadcast instead of a
    DMA bounce through DRAM.
  - y tiles stay in SBUF (bf16) and feed the projection directly; the
    projection emits per 512-token chunk interleaved with attention.
Host sums the 4 head-shard partial outputs (bf16) per batch and
transposes back.

Self-contained: shapes/sharding hardcoded; inputs full-size numpy arrays.
"""

import math
import os
import sys
import types

import numpy as np
import ml_dtypes

import concourse.bass as bass
import concourse.mybir as mybir
import concourse.tile as tile
from concourse import bacc
from concourse.bass_utils import run_bass_kernel_spmd

F32 = mybir.dt.float32
BF16 = mybir.dt.bfloat16
AF = mybir.ActivationFunctionType
ALU = mybir.AluOpType

# Problem shape (hardcoded per contract)
B, T, C = 4, 2048, 2048
H, HD = 16, 128
NCORES = 8
BGROUPS, HSHARDS = 2, 4  # batch groups x head shards
B_LOC = B // BGROUPS  # 2 batches per core
H_LOC = H // HSHARDS  # 4 heads per core
FQK = H_LOC * HD  # 512 features for q (and for k)
FV = H_LOC * HD  # 512 features for v
F_ALL = 3 * H_LOC * HD  # 1536 qkv features per core
KO = C // 128  # 16 contraction chunks
TSLAB = 512
NSLAB = T // TSLAB  # 4 t-slabs per batch
QTILE = 512
NQT = T // QTILE  # 4 q-tiles
NKB = T // 128  # 16 key blocks
SCALE = 1.0 / math.sqrt(HD)

_CACHED = {}


def _install_ntff_hook():
    """Register the axon NTFF profile hook (container's antenv lacks it)."""
    if "antenv.axon_hooks" in sys.modules:
        return
    try:
        mod = types.ModuleType("antenv.axon_hooks")
        holder = [None]
        mod.set_axon_ntff_profile_hook = lambda h: holder.__setitem__(0, h)
        mod.get_axon_ntff_profile_hook = lambda: holder[0]
        sys.modules["antenv.axon_hooks"] = mod
        import antenv

        antenv.axon_hooks = mod
        if "/root/.axon_site" not in sys.path:
            sys.path.insert(0, "/root/.axon_site")
        from trn_agent_boot.trn_boot import _ntff_profile_via_ctypes

        mod.set_axon_ntff_profile_hook(
            _ntff_profile_via_ctypes("/opt/axon/libaxon_pjrt.so")
        )
    except Exception:
        sys.modules.pop("antenv.axon_hooks", None)


def rope_perm_matrix():
    """lhsT for the rotate-half matmul: rot^T = PT.T @ q^T.
    rot[2i] = -q[2i+1], rot[2i+1] = q[2i]."""
    pt = np.zeros((HD, HD), dtype=np.float32)
    for i in range(HD // 2):
        pt[2 * i + 1, 2 * i] = -1.0
        pt[2 * i, 2 * i + 1] = 1.0
    return pt


def build_nc():
    nc = bacc.Bacc("TRN2", target_bir_lowering=False, debug=False)

    x_t = nc.dram_tensor("x_t", [B_LOC, C, T], BF16, kind="ExternalInput").ap()
    w_qkv = nc.dram_tensor("w_qkv", [C, F_ALL], BF16, kind="ExternalInput").ap()
    w_proj = nc.dram_tensor("w_proj", [FV, C], BF16, kind="ExternalInput").ap()
    sin_t = nc.dram_tensor("sin_t", [HD, T], BF16, kind="ExternalInput").ap()
    cos_t = nc.dram_tensor("cos_t", [HD, T], BF16, kind="ExternalInput").ap()
    pt = nc.dram_tensor("pt", [HD, HD], BF16, kind="ExternalInput").ap()
    ones_col = nc.dram_tensor("ones_col", [128, 1], BF16, kind="ExternalInput").ap()
    out_t = nc.dram_tensor("out_t", [B_LOC, C, T], BF16, kind="ExternalOutput").ap()

    with tile.TileContext(nc) as tc:
        with nc.allow_low_precision(reason="bf16 matmul pipeline by design"):
            _emit(nc, tc, x_t, w_qkv, w_proj, sin_t, cos_t, pt, ones_col, out_t)
    nc.compile()
    return nc


def _emit(nc, tc, x_t, w_qkv, w_proj, sin_t, cos_t, pt, ones_col, out_t):
    with (
        tc.tile_pool(name="dram", bufs=1, space="DRAM") as dram_pool,
        tc.tile_pool(name="consts", bufs=1) as consts,
        tc.tile_pool(name="wq", bufs=1) as wqpool,
        tc.tile_pool(name="wp", bufs=1) as wppool,
        tc.tile_pool(name="sc", bufs=1) as scpool,
        tc.tile_pool(name="xpool", bufs=2) as xpool,
        tc.tile_pool(name="rope", bufs=2) as ropepool,
        tc.tile_pool(name="qkvload", bufs=2) as qkvload,
        tc.tile_pool(name="vload", bufs=2) as vload,
        tc.tile_pool(name="ppool", bufs=6) as ppool,
        tc.tile_pool(name="ypool", bufs=2 * H_LOC) as ypool,
        tc.tile_pool(name="npool", bufs=2) as npool,
        tc.tile_pool(name="opool", bufs=3) as opool,
        tc.tile_pool(name="accum", bufs=3, space="PSUM") as accum,
        tc.tile_pool(name="spsum", bufs=2, space="PSUM") as spsum,
        tc.tile_pool(name="ypsum", bufs=2, space="PSUM") as ypsum,
        tc.tile_pool(name="lpsum", bufs=1, space="PSUM") as lpsum,
    ):
        qk_dram = [
            dram_pool.tile([2 * FQK, T], BF16, name=f"qk_dram{b}")
            for b in range(B_LOC)
        ]
        v_dram = [
            dram_pool.tile([T, FV], BF16, name=f"v_dram{b}") for b in range(B_LOC)
        ]

        pt_sb = consts.tile([HD, HD], BF16)
        nc.sync.dma_start(pt_sb, pt)
        ones_c_sb = consts.tile([128, 1], BF16)
        nc.sync.dma_start(ones_c_sb, ones_col)

        # warm-up: the PE clock gate ramps 1.2->2.4GHz only after ~3.4us of
        # sustained matmul activity; burn dummy matmuls on a zeroed scratch
        # tile while the first weight/x chunks stream in.
        warm_sb = consts.tile([128, TSLAB], BF16)
        nc.gpsimd.memset(warm_sb, 0)
        for _ in range(18):
            warm_ps = accum.tile([128, TSLAB], F32, name="warm_ps", tag="acc")
            nc.tensor.matmul(warm_ps, warm_sb[:, :128], warm_sb, start=True, stop=True)
        # w chunks are interleaved with the first x slab's chunks below so
        # the first matmul starts after ~1/16 of each load; wp loads at the
        # end of stage 1 (first needed by proj in stage 2).
        w_sb = wqpool.tile([128, KO, F_ALL], BF16)
        w_src = w_qkv.rearrange("(ko p) f -> p ko f", p=128)
        wp_sb = wppool.tile([128, H_LOC, C], BF16)
        sin_sb = scpool.tile([HD, T], BF16)
        cos_sb = scpool.tile([HD, T], BF16)

        # ---------- qkv phase: one t-slab of 512 tokens ----------
        def emit_qkv_slab(b, js, first=False):
            tsl = slice(js * TSLAB, (js + 1) * TSLAB)
            x3 = x_t[b].rearrange("(ko p) t -> p ko t", p=128)
            x_sb = xpool.tile([128, KO, TSLAB], BF16, name="x_sb")
            if first:
                # interleave weight and x chunk loads so the ko=0 matmul
                # starts after ~1/16 of each tensor has landed
                for ko in range(KO):
                    nc.sync.dma_start(w_sb[:, ko, :], w_src[:, ko, :])
                    nc.sync.dma_start(x_sb[:, ko, :], x3[:, ko, tsl])
                    if ko == 0:
                        nc.sync.dma_start(sin_sb, sin_t)
                        nc.sync.dma_start(cos_sb, cos_t)
            else:
                nc.sync.dma_start(x_sb, x3[:, :, tsl])

            # q^T, k^T feature chunks (one head each) with RoPE.
            # rot-matmul+combine for chunk f is emitted one chunk late so
            # the tensor queue never waits on the raw-copy cast.
            pending = None  # (raw_bf, f)

            def flush_rope():
                raw_bf, f = pending
                rot_ps = accum.tile([128, TSLAB], F32, name="rot_ps", tag="acc")
                nc.tensor.matmul(rot_ps, pt_sb, raw_bf, start=True, stop=True)
                t1 = ropepool.tile([128, TSLAB], BF16, name="t1")
                nc.vector.tensor_tensor(t1, raw_bf, cos_sb[:, tsl], ALU.mult)
                t2 = ropepool.tile([128, TSLAB], BF16, name="t2")
                nc.vector.tensor_tensor(t2, rot_ps, sin_sb[:, tsl], ALU.mult)
                roped = ropepool.tile([128, TSLAB], BF16, name="roped")
                nc.vector.tensor_tensor(roped, t1, t2, ALU.add)
                nc.sync.dma_start(qk_dram[b][f * 128 : (f + 1) * 128, tsl], roped)

            for f in range(2 * H_LOC):
                ps = accum.tile([128, TSLAB], F32, name="qk_ps", tag="acc")
                for ko in range(KO):
                    nc.tensor.matmul(
                        ps,
                        w_sb[:, ko, f * 128 : (f + 1) * 128],
                        x_sb[:, ko, :],
                        start=(ko == 0),
                        stop=(ko == KO - 1),
                    )
                raw_bf = ropepool.tile([128, TSLAB], BF16, name="raw_bf", bufs=3)
                nc.vector.tensor_copy(raw_bf, ps)
                if pending is not None:
                    flush_rope()
                pending = (raw_bf, f)

            # v in natural [t, f] layout
            for tb in range(TSLAB // 128):
                vps = accum.tile([128, FV], F32, name="v_ps", tag="acc")
                for ko in range(KO):
                    nc.tensor.matmul(
                        vps,
                        x_sb[:, ko, tb * 128 : (tb + 1) * 128],
                        w_sb[:, ko, 2 * FQK : 2 * FQK + FV],
                        start=(ko == 0),
                        stop=(ko == KO - 1),
                    )
                if pending is not None:
                    flush_rope()
                    pending = None
                v_bf = ropepool.tile([128, FV], BF16, name="v_bf", bufs=3)
                nc.vector.tensor_copy(v_bf, vps)
                r0 = js * TSLAB + tb * 128
                nc.sync.dma_start(v_dram[b][r0 : r0 + 128, :], v_bf)

        # ---------- attention ----------
        head_loads = {}
        vpair_loads = {}

        def emit_head_load(b, h):
            qt_sb = qkvload.tile([HD, T], BF16, name="qt_sb")
            nc.sync.dma_start(qt_sb, qk_dram[b][h * HD : (h + 1) * HD, :])
            kt_sb = qkvload.tile([HD, T], BF16, name="kt_sb")
            nc.sync.dma_start(
                kt_sb, qk_dram[b][FQK + h * HD : FQK + (h + 1) * HD, :]
            )
            head_loads[(b, h)] = (qt_sb, kt_sb)
            if h % 2 == 0:
                v_sb = vload.tile([128, NKB, 2 * HD], BF16, name="v_sb")
                nc.sync.dma_start(
                    v_sb,
                    v_dram[b].rearrange("(kb p) f -> p kb f", p=128)[
                        :, :, h * HD : (h + 2) * HD
                    ],
                )
                vpair_loads[(b, h // 2)] = v_sb

        bh_order = [(b, h) for b in range(B_LOC) for h in range(H_LOC)]

        def emit_attn_head(b, h):
            # prefetch next head's q/k/v
            i = bh_order.index((b, h))
            if i + 1 < len(bh_order):
                nb, nh = bh_order[i + 1]
                if (nb, nh) not in head_loads:
                    emit_head_load(nb, nh)
            qt_sb, kt_sb = head_loads.pop((b, h))
            v_sb = vpair_loads[(b, h // 2)]
            vc = (h % 2) * HD
            y_sb = ypool.tile([HD, T], BF16, name="y_sb")
            norm_pairs = []
            for jq in range(NQT):
                qsl = slice(jq * QTILE, (jq + 1) * QTILE)
                nkb = 4 * (jq + 1)
                y_ps = ypsum.tile([HD, QTILE], F32, name="y_ps")
                l_ps = lpsum.tile([1, QTILE], F32, name="l_ps")

                # software-pipelined blocks: the score matmul for block kb+1
                # is emitted before the l/y matmuls of block kb, so the
                # tensor queue never waits head-of-line on the exp output.
                def emit_score(kb):
                    s_diag = kb - 4 * jq
                    qoff = 128 * s_diag if s_diag > 0 else 0
                    qsub = slice(jq * QTILE + qoff, (jq + 1) * QTILE)
                    s_ps = spsum.tile([128, QTILE], F32, name="s_ps")
                    nc.tensor.matmul(
                        s_ps[:, qoff:],
                        kt_sb[:, kb * 128 : (kb + 1) * 128],
                        qt_sb[:, qsub],
                        start=True,
                        stop=True,
                    )
                    p_bf = ppool.tile([128, QTILE], BF16, name="p_bf")
                    nc.scalar.activation(
                        p_bf[:, qoff:], s_ps[:, qoff:], AF.Exp, scale=SCALE
                    )
                    if s_diag >= 0:
                        # causal: mask only the diagonal 128x128 sub-block
                        # (columns right of it always satisfy q > k)
                        nc.gpsimd.affine_select(
                            out=p_bf[:, qoff : qoff + 128],
                            in_=p_bf[:, qoff : qoff + 128],
                            pattern=[[1, 128]],
                            compare_op=ALU.is_ge,
                            fill=0.0,
                            base=0,
                            channel_multiplier=-1,
                        )
                    return p_bf, qoff

                def emit_ly(kb, p_bf, qoff):
                    nc.tensor.matmul(
                        l_ps[:, qoff:],
                        ones_c_sb,
                        p_bf[:, qoff:],
                        start=(kb == 0),
                        stop=(kb == nkb - 1),
                    )
                    nc.tensor.matmul(
                        y_ps[:, qoff:],
                        v_sb[:, kb, vc : vc + HD],
                        p_bf[:, qoff:],
                        start=(kb == 0),
                        stop=(kb == nkb - 1),
                    )

                prev = emit_score(0)
                for kb in range(1, nkb):
                    cur = emit_score(kb)
                    emit_ly(kb - 1, *prev)
                    prev = cur
                emit_ly(nkb - 1, *prev)

                linv = npool.tile([1, QTILE], F32, name="linv")
                nc.vector.reciprocal_approx_fast(linv, l_ps)
                linv_bf = npool.tile([1, QTILE], BF16, name="linv_bf")
                nc.vector.tensor_copy(linv_bf, linv)
                bc = npool.tile([128, QTILE], BF16, name="bc", bufs=4)
                nc.gpsimd.partition_broadcast(bc, linv_bf)
                # evacuate unnormalized (frees the psum bank); normalize at
                # head end once the broadcast is surely done, so the DVE
                # queue never waits head-of-line on the recip/bc chain.
                nc.vector.tensor_copy(y_sb[:, qsl], y_ps)
                norm_pairs.append((qsl, bc))
            for qsl_n, bc_n in norm_pairs:
                nc.vector.tensor_tensor(
                    y_sb[:, qsl_n], y_sb[:, qsl_n], bc_n, ALU.mult
                )
            return y_sb

        # ---------- output projection, one 512-token chunk ----------
        def emit_proj_chunk(b, jt, y_tiles, evac_on_scalar=False):
            tsl = slice(jt * QTILE, (jt + 1) * QTILE)
            for co in range(C // 128):
                o_ps = accum.tile([128, QTILE], F32, name="o_ps", tag="acc")
                for h in range(H_LOC):
                    nc.tensor.matmul(
                        o_ps,
                        wp_sb[:, h, co * 128 : (co + 1) * 128],
                        y_tiles[h][:, tsl],
                        start=(h == 0),
                        stop=(h == H_LOC - 1),
                    )
                o_bf = opool.tile([128, QTILE], BF16, name="o_bf")
                if evac_on_scalar:
                    # ACT is idle in stage 3; keeps the DVE queue off the
                    # o_ps psum-ring critical path
                    nc.scalar.activation(o_bf, o_ps, AF.Copy)
                else:
                    nc.vector.tensor_copy(o_bf, o_ps)
                nc.sync.dma_start(out_t[b, co * 128 : (co + 1) * 128, tsl], o_bf)

        # ---------- fused schedule ----------
        y_by_batch = {0: [], 1: []}
        # stage 0: qkv(b0)
        for js in range(NSLAB):
            emit_qkv_slab(0, js, first=(js == 0))
        # stage 1: qkv(b1) slabs interleaved with attn(b0) heads
        for i in range(NSLAB):
            emit_qkv_slab(1, i)
            if i == 0:
                emit_head_load(0, 0)
            y_by_batch[0].append(emit_attn_head(0, i))
        nc.sync.dma_start(wp_sb, w_proj.rearrange("(fo p) c -> p fo c", p=128))
        # stage 2: attn(b1) heads interleaved with proj(b0) chunks
        for i in range(H_LOC):
            y_by_batch[1].append(emit_attn_head(1, i))
            emit_proj_chunk(0, i, y_by_batch[0])
        # stage 3: proj(b1)
        for jt in range(NQT):
            emit_proj_chunk(1, jt, y_by_batch[1], evac_on_scalar=True)


def _get_nc():
    if "nc" not in _CACHED:
        _CACHED["nc"] = build_nc()
    return _CACHED["nc"]


def kernel(x, sin, cos, W_qkv, W_proj):
    x = np.asarray(x, dtype=np.float32)
    sin = np.asarray(sin, dtype=np.float32)
    cos = np.asarray(cos, dtype=np.float32)
    W_qkv = np.asarray(W_qkv, dtype=np.float32)
    W_proj = np.asarray(W_proj, dtype=np.float32)

    bf = ml_dtypes.bfloat16
    sin_t = np.ascontiguousarray(sin[0, 0].T).astype(bf)  # [HD, T]
    cos_t = np.ascontiguousarray(cos[0, 0].T).astype(bf)
    pt = rope_perm_matrix().astype(bf)
    ones_col = np.ones((128, 1), bf)

    in_maps = []
    for g in range(BGROUPS):
        x_tg = np.ascontiguousarray(
            x[g * B_LOC : (g + 1) * B_LOC].transpose(0, 2, 1)
        ).astype(bf)  # [B_LOC, C, T]
        for s in range(HSHARDS):
            qcols = W_qkv[:, s * FQK : (s + 1) * FQK]
            kcols = W_qkv[:, C + s * FQK : C + (s + 1) * FQK]
            vcols = W_qkv[:, 2 * C + s * FV : 2 * C + (s + 1) * FV]
            w_qkv_loc = np.ascontiguousarray(
                np.concatenate([qcols, kcols, vcols], axis=1)
            ).astype(bf)
            w_proj_loc = np.ascontiguousarray(
                W_proj[s * FV : (s + 1) * FV, :]
            ).astype(bf)
            in_maps.append(
                {
                    "x_t": x_tg,
                    "w_qkv": w_qkv_loc,
                    "w_proj": w_proj_loc,
                    "sin_t": sin_t,
                    "cos_t": cos_t,
                    "pt": pt,
                    "ones_col": ones_col,
                }
            )

    trace = bool(int(os.environ.get("KERNEL_TRACE", "0")))
    if trace:
        _install_ntff_hook()
    nc = _get_nc()
    res = run_bass_kernel_spmd(
        nc, in_maps, core_ids=list(range(NCORES)), trace=trace
    )
    _CACHED["last_result"] = res

    out = np.zeros((B, T, C), dtype=np.float32)
    for g in range(BGROUPS):
        acc = np.zeros((B_LOC, C, T), dtype=np.float32)
        for s in range(HSHARDS):
            acc += np.asarray(res.results[g * HSHARDS + s]["out_t"], dtype=np.float32)
        out[g * B_LOC : (g + 1) * B_LOC] = acc.transpose(0, 2, 1)
    return out
